# revision 1
# baseline (speedup 1.0000x reference)
"""Trainium2 Bass kernel v3 for batched 22-node complete-digraph GNN.

Per-graph sum restructured: lift bias folded into the matmul (ones-row in
the feature tile), then per chunk
  step1 (Act):  xh = relu(psum[:, g, 11:22])            # 704 cols
  step2 (DVE):  t11[:, g, 0:11] = max(psum[:, g, 0:11], 0) + xh   # STT
and stage-2's first matmul accumulates the 11 partials in PSUM:
  h1 = sum_k W1blk^T @ t11[:, :, k].
"""

import os

import numpy as np

B = 32768
G = 22
N = B * G
NCORES = 8
BC = B // NCORES          # 4096 graphs per core
NC_NODES = BC * G         # 90112 nodes per core
NBLK = 8
CHUNK_NODES = NC_NODES // NBLK   # 11264 node-cols
FT_R = 10                 # 9 features + ones row per slab
FT_P = FT_R * NBLK        # 80
LIFT_P = 15 * NBLK        # 120
TOTG = CHUNK_NODES // G   # 512 graph-cols

CHUNKS = [int(c) for c in os.environ.get(
    "KERNEL_CHUNKS",
    "572,1012,1012,1012,1012,1012,1012,1012,1012,1012,1012,572").split(",")]
assert sum(CHUNKS) == CHUNK_NODES and all(c % G == 0 for c in CHUNKS)
assert all(c <= 1012 for c in CHUNKS)   # psum tile is [120, 1024] f32 = 2 banks
NCHUNK = len(CHUNKS)
CSTART = [sum(CHUNKS[:i]) for i in range(NCHUNK + 1)]

# ft DMA segment widths (first rides with the head DMA)
DMASEG = [int(x) for x in os.environ.get(
    "KERNEL_DMASEG", "572,1012,2024,3036,3036,1584").split(",")]
assert sum(DMASEG) == CHUNK_NODES
DSTART = [sum(DMASEG[:i]) for i in range(len(DMASEG) + 1)]

WPACK_COLS = 256
NWARM = int(os.environ.get("KERNEL_WARM", "2"))
SEGS = [int(x) for x in os.environ.get("KERNEL_SEGS", "0,128,256,384,486,512").split(",")]
# stage2 activation engine per segment: A=Act, D=DVE
S2ENG = os.environ.get("KERNEL_S2ENG", "AAADD")
S2LAG = int(os.environ.get("KERNEL_S2LAG", "3"))
SPLIT = int(os.environ.get("KERNEL_SPLIT", "11"))   # stt pairs cols [0:S] with [S:2S]
XHW = G - SPLIT                                     # relu-evac width (Act)
NPART = G - SPLIT                                   # partials per graph for h1-accum

LAST_RESULT = None


def _structured(src, dst):
    offsets = np.repeat(np.arange(B, dtype=np.int32) * G, G * G)
    ls = np.tile(np.repeat(np.arange(G, dtype=np.int32), G), B)
    ld = np.tile(np.tile(np.arange(G, dtype=np.int32), G), B)
    return np.array_equal(src, offsets + ls) and np.array_equal(dst, offsets + ld)


def _fallback_numpy(features, src, dst, W_lift, b_lift, W1, b1, W2, b2, W_ro, b_ro):
    x = np.maximum(features @ W_lift + b_lift, 0.0)
    agg = np.zeros((N, x.shape[1]), np.float32)
    np.add.at(agg, dst, x[src])
    x = np.maximum(agg @ W1 + b1, 0.0)
    agg = np.zeros((N, x.shape[1]), np.float32)
    np.add.at(agg, dst, x[src])
    x = np.maximum(agg @ W2 + b2, 0.0)
    x = x.reshape(B, G * 5)
    return (x @ W_ro + b_ro).astype(np.float32)


def _block_diag(W, nblk, dtype):
    fi, fo = W.shape
    out = np.zeros((fi * nblk, fo * nblk), dtype)
    for a in range(nblk):
        out[fi * a:fi * (a + 1), fo * a:fo * (a + 1)] = W
    return out


_cached = {}


def _build_kernel():
    import concourse.bacc as bacc
    import concourse.mybir as mybir
    from concourse.tile import TileContext

    f16 = mybir.dt.float16
    f32 = mybir.dt.float32
    Relu = mybir.ActivationFunctionType.Relu
    Add = mybir.AluOpType.add
    Max = mybir.AluOpType.max

    nc = bacc.Bacc(trn_type="TRN2", enable_partition_id=False)

    head_cols = WPACK_COLS + DMASEG[0]
    head_d = nc.dram_tensor("head", [LIFT_P, head_cols], f16,
                            kind="ExternalInput")
    ft_d = nc.dram_tensor("ft", [FT_P, CHUNK_NODES - DMASEG[0]], f16,
                          kind="ExternalInput")
    out_d = nc.dram_tensor("out", [NBLK, TOTG], f32, kind="ExternalOutput")

    with TileContext(nc) as tc:
        with (
            tc.tile_pool(name="consts", bufs=1) as consts,
            tc.tile_pool(name="ft", bufs=1) as ftp,
            tc.tile_pool(name="tree", bufs=1) as treep,
            tc.tile_pool(name="s2", bufs=1) as s2p,
            tc.tile_pool(name="lift_ps", bufs=3, space="PSUM") as psp,
            tc.tile_pool(name="s2_ps", bufs=2, space="PSUM") as ps2p,
        ):
            # ---- one head DMA: weights + ft chunk 0
            head_sb = consts.tile([LIFT_P, head_cols], f16, name="head")
            nc.sync.dma_start(out=head_sb, in_=head_d[:, :])
            wp_sb = head_sb[:, 0:WPACK_COLS]
            ft_tiles = [head_sb[0:FT_P, WPACK_COLS:head_cols]]
            for d in range(1, len(DMASEG)):
                t = ftp.tile([FT_P, DMASEG[d]], f16, tag=f"ft{d}",
                             name=f"ft_sb{d}")
                nc.sync.dma_start(
                    out=t, in_=ft_d[:, DSTART[d] - DMASEG[0]:
                                    DSTART[d + 1] - DMASEG[0]])
                ft_tiles.append(t)

            def ft_slice(lo, hi):
                """(tile, tlo, thi) pieces covering ft cols [lo, hi)."""
                out = []
                for d in range(len(DMASEG)):
                    s, e = DSTART[d], DSTART[d + 1]
                    a, b = max(lo, s), min(hi, e)
                    if a < b:
                        out.append((ft_tiles[d], a - s, b - s))
                return out

            # ---- PE p-state warm-up + early Relu act-table load
            warm_sb = consts.tile([FT_P, 128], f16)
            nc.vector.memset(warm_sb, 0.0)
            bias0 = consts.tile([128, 1], f32)
            nc.vector.memset(bias0, 0.0)
            relu_warm = consts.tile([128, 1], f16)
            nc.scalar.activation(out=relu_warm, in_=bias0[:, 0:1],
                                 func=Relu, bias=bias0[:, 0:1], scale=1.0)
            warm_ps = ps2p.tile([LIFT_P, 128], f32, tag="s2", bufs=2,
                                name="warmps")
            for _ in range(NWARM):
                nc.tensor.matmul(warm_ps[:, :], lhsT=warm_sb[:, 0:LIFT_P],
                                 rhs=warm_sb[:, :], start=True, stop=True)

            wlift_sb = wp_sb[0:FT_P, 0:LIFT_P]
            w1_sb = wp_sb[:, 120:200]          # [120, 80]
            w2_sb = wp_sb[0:10 * NBLK, 200:240]
            wro_sb = wp_sb[0:5 * NBLK, 240:248]
            bias_f32 = wp_sb[:, 248:256].bitcast(f32)   # [120, 4]
            b1 = bias_f32[0:10 * NBLK, 1:2]
            b2 = bias_f32[0:5 * NBLK, 2:3]
            bro = bias_f32[0:NBLK, 3:4]

            # absorb the wpack-DMA wait once per consumer engine
            dummy = consts.tile([LIFT_P, 4], f32)
            nc.scalar.copy(out=dummy, in_=bias_f32[:, 0:4])
            dummy2 = consts.tile([LIFT_P, 1], f32)
            nc.vector.tensor_copy(out=dummy2, in_=bias_f32[:, 0:1])

            t11 = treep.tile([LIFT_P, TOTG, SPLIT], f16, name="t11")
            xh12 = treep.tile([LIFT_P, TOTG, XHW], f16, name="xh12")

            def lift_chunk(c):
                ps = psp.tile([LIFT_P, 1024], f32, tag="ps", name=f"ps_{c}")
                off = 0
                for (t, tlo, thi) in ft_slice(CSTART[c], CSTART[c + 1]):
                    nc.tensor.ldweights(weights=t[:, tlo:min(thi, tlo + 128)])
                    col = tlo
                    while col < thi:
                        n = min(512, thi - col)
                        nc.tensor.matmul(
                            ps[:, off:off + n],
                            lhsT=wlift_sb, rhs=t[:, col:col + n],
                            start=True, stop=True)
                        col += n
                        off += n
                return ps

            def reduce_chunk(c, ps):
                w = CHUNKS[c]
                ng = w // G
                g0 = CSTART[c] // G
                ps3 = ps[:, 0:w].rearrange("p (g i) -> p g i", i=G)
                nc.scalar.activation(
                    out=xh12[:, g0:g0 + ng, :], in_=ps3[:, :, SPLIT:G],
                    func=Relu, bias=0.0, scale=1.0)
                with nc.allow_low_precision(reason="fp16 partial sums"):
                    nc.vector.scalar_tensor_tensor(
                        out=t11[:, g0:g0 + ng, :], in0=ps3[:, :, 0:SPLIT],
                        scalar=0.0, in1=xh12[:, g0:g0 + ng, 0:SPLIT],
                        op0=Max, op1=Add)


            # ---- stage-2 pieces, interleaved into the chunk loop
            o_sb = s2p.tile([NBLK, TOTG], f32, name="osb")
            s2_state = {}

            def s2_p1(k):
                lo, hi = SEGS[k], SEGS[k + 1]
                w = hi - lo
                eng = S2ENG[min(k, len(S2ENG) - 1)]
                h1_ps = ps2p.tile([10 * NBLK, 512], f32, tag="s2",
                                  name=f"h1ps_{k}")
                for j in range(NPART):
                    rhs = (t11[:, lo:hi, j] if j < SPLIT
                           else xh12[:, lo:hi, j])
                    nc.tensor.matmul(h1_ps[:, 0:w], lhsT=w1_sb, rhs=rhs,
                                     start=(j == 0), stop=(j == NPART - 1))
                h1_sb = s2p.tile([10 * NBLK, 512], f16, tag=f"h1_{k}",
                                 name=f"h1sb_{k}")
                if eng == "A":
                    nc.scalar.activation(out=h1_sb[:, 0:w], in_=h1_ps[:, 0:w],
                                         func=Relu, bias=b1, scale=1.0)
                else:
                    nc.vector.tensor_scalar(
                        out=h1_sb[:, 0:w], in0=h1_ps[:, 0:w],
                        scalar1=b1, scalar2=0.0, op0=Add, op1=Max)
                s2_state[k] = h1_sb

            def s2_p2(k):
                lo, hi = SEGS[k], SEGS[k + 1]
                w = hi - lo
                eng = S2ENG[min(k, len(S2ENG) - 1)]
                h1_sb = s2_state[k]
                h2_ps = ps2p.tile([5 * NBLK, 512], f32, tag="s2",
                                  name=f"h2ps_{k}")
                nc.tensor.matmul(h2_ps[:, 0:w], lhsT=w2_sb, rhs=h1_sb[:, 0:w],
                                 start=True, stop=True)
                h2_sb = s2p.tile([5 * NBLK, 512], f16, tag=f"h2_{k}",
                                 name=f"h2sb_{k}")
                if eng == "A":
                    nc.scalar.activation(out=h2_sb[:, 0:w], in_=h2_ps[:, 0:w],
                                         func=Relu, bias=b2, scale=1.0)
                else:
                    nc.vector.tensor_scalar(
                        out=h2_sb[:, 0:w], in0=h2_ps[:, 0:w],
                        scalar1=b2, scalar2=0.0, op0=Add, op1=Max)
                s2_state[k] = h2_sb

            def s2_p3(k):
                lo, hi = SEGS[k], SEGS[k + 1]
                w = hi - lo
                h2_sb = s2_state[k]
                o_ps = ps2p.tile([NBLK, 512], f32, tag="s2", name=f"ops_{k}")
                nc.tensor.matmul(o_ps[:, 0:w], lhsT=wro_sb, rhs=h2_sb[:, 0:w],
                                 start=True, stop=True)
                nc.vector.tensor_scalar(out=o_sb[:, lo:hi], in0=o_ps[:, 0:w],
                                        scalar1=bro, scalar2=None, op0=Add)

            # chunk index after which each piece may be emitted
            gcum = 0
            lastch = []   # last chunk index for each segment
            cg = [0]
            for c in CHUNKS:
                gcum += c // G
                cg.append(gcum)
            for k in range(len(SEGS) - 1):
                need = SEGS[k + 1]
                lc = next(i for i in range(NCHUNK) if cg[i + 1] >= need)
                lastch.append(lc)
            pieces = []
            for k in range(len(SEGS) - 1):
                pieces.append((lastch[k] + S2LAG, 1, k))
                pieces.append((lastch[k] + S2LAG + 1, 2, k))
                pieces.append((lastch[k] + S2LAG + 2, 3, k))
            pieces.sort()
            s2fn = {1: s2_p1, 2: s2_p2, 3: s2_p3}

            pi = 0
            for c in range(NCHUNK):
                ps = lift_chunk(c)
                reduce_chunk(c, ps)
                while pi < len(pieces) and pieces[pi][0] <= c:
                    s2fn[pieces[pi][1]](pieces[pi][2])
                    pi += 1
            while pi < len(pieces):
                s2fn[pieces[pi][1]](pieces[pi][2])
                pi += 1
            nc.sync.dma_start(out=out_d[:, :], in_=o_sb[:, :])

    if not nc.is_finalized():
        nc.finalize()
    return nc


def kernel(features, src, dst, W_lift, b_lift, W1, b1, W2, b2, W_ro, b_ro):
    global LAST_RESULT
    features = np.asarray(features, np.float32)
    src = np.asarray(src, np.int32)
    dst = np.asarray(dst, np.int32)
    W_lift = np.asarray(W_lift, np.float32)
    b_lift = np.asarray(b_lift, np.float32)
    W1 = np.asarray(W1, np.float32)
    b1 = np.asarray(b1, np.float32)
    W2 = np.asarray(W2, np.float32)
    b2 = np.asarray(b2, np.float32)
    W_ro = np.asarray(W_ro, np.float32)
    b_ro = np.asarray(b_ro, np.float32)

    if not _structured(src, dst):
        return _fallback_numpy(features, src, dst, W_lift, b_lift,
                               W1, b1, W2, b2, W_ro, b_ro)

    # feature-major per-slab layout with a ones row per slab: [80, 11264]
    ftn = (features.reshape(NCORES, NBLK, CHUNK_NODES, 9)
           .transpose(0, 1, 3, 2))           # [NCORES, NBLK, 9, CHUNK_NODES]
    ft = np.ones((NCORES, NBLK, FT_R, CHUNK_NODES), np.float16)
    ft[:, :, 0:9, :] = ftn.astype(np.float16)
    ft = ft.reshape(NCORES, FT_P, CHUNK_NODES)

    wpack = np.zeros((LIFT_P, WPACK_COLS), np.float16)
    wla = np.concatenate([W_lift, b_lift[None, :]], axis=0)   # [10, 15]
    wpack[0:FT_P, 0:LIFT_P] = _block_diag(wla, NBLK, np.float16)
    wpack[0:LIFT_P, 120:200] = _block_diag(W1, NBLK, np.float16)
    wpack[0:10 * NBLK, 200:240] = _block_diag((G * W2).astype(np.float32),
                                              NBLK, np.float16)
    wro_eff = W_ro.reshape(G, 5).sum(axis=0)
    for a in range(NBLK):
        wpack[5 * a:5 * (a + 1), 240 + a] = wro_eff

    bpack = np.zeros((LIFT_P, 4), np.float32)
    bpack[0:10 * NBLK, 1] = np.tile(b1, NBLK)
    bpack[0:5 * NBLK, 2] = np.tile(b2, NBLK)
    bpack[0:NBLK, 3] = float(b_ro[0])
    wpack[:, 248:256] = bpack.view(np.float16)

    if "nc" not in _cached:
        _cached["nc"] = _build_kernel()
    nc = _cached["nc"]

    from concourse import bass_utils

    hn = DMASEG[0]
    in_maps = []
    for c in range(NCORES):
        head = np.zeros((LIFT_P, WPACK_COLS + hn), np.float16)
        head[:, 0:WPACK_COLS] = wpack
        head[0:FT_P, WPACK_COLS:] = ft[c, :, 0:hn]
        in_maps.append({
            "head": head,
            "ft": np.ascontiguousarray(ft[c, :, hn:]),
        })

    trace = os.environ.get("KERNEL_TRACE", "0") == "1"
    res = None
    for attempt in range(4):
        try:
            res = bass_utils.run_bass_kernel_spmd(
                nc, in_maps, core_ids=list(range(NCORES)), trace=trace,
            )
            break
        except ModuleNotFoundError:
            trace = False
        except Exception as e:  # noqa: BLE001
            if attempt == 3 or "UNRECOVERABLE" not in str(e).upper():
                raise
            import time
            time.sleep(15)
    LAST_RESULT = res

    out = np.concatenate([r["out"].reshape(-1) for r in res.results])
    return np.ascontiguousarray(out.reshape(B, 1).astype(np.float32))



# revision 2
# speedup vs baseline: 1.0010x; 1.0010x over previous
"""Trainium2 Bass kernel v3 for batched 22-node complete-digraph GNN.

Per-graph sum restructured: lift bias folded into the matmul (ones-row in
the feature tile), then per chunk
  step1 (Act):  xh = relu(psum[:, g, 11:22])            # 704 cols
  step2 (DVE):  t11[:, g, 0:11] = max(psum[:, g, 0:11], 0) + xh   # STT
and stage-2's first matmul accumulates the 11 partials in PSUM:
  h1 = sum_k W1blk^T @ t11[:, :, k].
"""

import os

import numpy as np

B = 32768
G = 22
N = B * G
NCORES = 8
BC = B // NCORES          # 4096 graphs per core
NC_NODES = BC * G         # 90112 nodes per core
NBLK = 8
CHUNK_NODES = NC_NODES // NBLK   # 11264 node-cols
FT_R = 10                 # 9 features + ones row per slab
FT_P = FT_R * NBLK        # 80
LIFT_P = 15 * NBLK        # 120
TOTG = CHUNK_NODES // G   # 512 graph-cols

CHUNKS = [int(c) for c in os.environ.get(
    "KERNEL_CHUNKS",
    "572,1012,1012,1012,1012,1012,1012,1012,1012,1012,1012,572").split(",")]
assert sum(CHUNKS) == CHUNK_NODES and all(c % G == 0 for c in CHUNKS)
assert all(c <= 1012 for c in CHUNKS)   # psum tile is [120, 1024] f32 = 2 banks
NCHUNK = len(CHUNKS)
CSTART = [sum(CHUNKS[:i]) for i in range(NCHUNK + 1)]

# ft DMA segment widths (first rides with the head DMA)
DMASEG = [int(x) for x in os.environ.get(
    "KERNEL_DMASEG", "572,1012,2024,3036,3036,1584").split(",")]
assert sum(DMASEG) == CHUNK_NODES
DSTART = [sum(DMASEG[:i]) for i in range(len(DMASEG) + 1)]

WPACK_COLS = 256
NWARM = int(os.environ.get("KERNEL_WARM", "2"))
OKV = os.environ.get("KERNEL_OKV", "0") == "1"
SEGS = [int(x) for x in os.environ.get("KERNEL_SEGS", "0,128,256,384,486,512").split(",")]
# stage2 activation engine per segment: A=Act, D=DVE
S2ENG = os.environ.get("KERNEL_S2ENG", "AAADD")
S2LAG = int(os.environ.get("KERNEL_S2LAG", "3"))
SPLIT = int(os.environ.get("KERNEL_SPLIT", "11"))   # stt pairs cols [0:S] with [S:2S]
TAILREV = os.environ.get("KERNEL_TAILREV", "0") == "1"
NFOLD = int(os.environ.get("KERNEL_NFOLD", "4"))    # pool folds t11 pairs
FOLD_SEGS = int(os.environ.get("KERNEL_FOLD_SEGS", "3"))  # segs 0..k-1 use folds
XHW = G - SPLIT                                     # relu-evac width (Act)
NPART = G - SPLIT                                   # partials per graph for h1-accum

LAST_RESULT = None


def _structured(src, dst):
    offsets = np.repeat(np.arange(B, dtype=np.int32) * G, G * G)
    ls = np.tile(np.repeat(np.arange(G, dtype=np.int32), G), B)
    ld = np.tile(np.tile(np.arange(G, dtype=np.int32), G), B)
    return np.array_equal(src, offsets + ls) and np.array_equal(dst, offsets + ld)


def _fallback_numpy(features, src, dst, W_lift, b_lift, W1, b1, W2, b2, W_ro, b_ro):
    x = np.maximum(features @ W_lift + b_lift, 0.0)
    agg = np.zeros((N, x.shape[1]), np.float32)
    np.add.at(agg, dst, x[src])
    x = np.maximum(agg @ W1 + b1, 0.0)
    agg = np.zeros((N, x.shape[1]), np.float32)
    np.add.at(agg, dst, x[src])
    x = np.maximum(agg @ W2 + b2, 0.0)
    x = x.reshape(B, G * 5)
    return (x @ W_ro + b_ro).astype(np.float32)


def _block_diag(W, nblk, dtype):
    fi, fo = W.shape
    out = np.zeros((fi * nblk, fo * nblk), dtype)
    for a in range(nblk):
        out[fi * a:fi * (a + 1), fo * a:fo * (a + 1)] = W
    return out


_cached = {}


def _build_kernel():
    import concourse.bacc as bacc
    import concourse.mybir as mybir
    from concourse.tile import TileContext

    f16 = mybir.dt.float16
    f32 = mybir.dt.float32
    Relu = mybir.ActivationFunctionType.Relu
    Add = mybir.AluOpType.add
    Max = mybir.AluOpType.max

    nc = bacc.Bacc(trn_type="TRN2", enable_partition_id=False,
                   num_swdge_queues=2)

    head_cols = WPACK_COLS + DMASEG[0]
    head_d = nc.dram_tensor("head", [LIFT_P, head_cols], f16,
                            kind="ExternalInput")
    ft_d = nc.dram_tensor("ft", [FT_P, CHUNK_NODES - DMASEG[0]], f16,
                          kind="ExternalInput")
    out_d = nc.dram_tensor("out", [NBLK, TOTG], f32, kind="ExternalOutput")

    with TileContext(nc) as tc:
        with (
            tc.tile_pool(name="consts", bufs=1) as consts,
            tc.tile_pool(name="ft", bufs=1) as ftp,
            tc.tile_pool(name="tree", bufs=1) as treep,
            tc.tile_pool(name="s2", bufs=1) as s2p,
            tc.tile_pool(name="lift_ps", bufs=3, space="PSUM") as psp,
            tc.tile_pool(name="s2_ps", bufs=2, space="PSUM") as ps2p,
        ):
            # ---- p-state pin: matmul ASAP so pe_busy_start ~ 0 and the
            # 3us ramp to full clock elapses during the DMA startup window
            tiny = consts.tile([2, 64], f16, name="tiny")
            nc.vector.memset(tiny, 0.0)
            tiny_ps = ps2p.tile([64, 512], f32, tag="s2", name="tinyps")
            for _ in range(NWARM):
                nc.tensor.matmul(tiny_ps[0:64, 0:64], lhsT=tiny[0:2, 0:64],
                                 rhs=tiny[0:2, 0:64], start=True, stop=True)

            # ---- one head DMA: weights + ft chunk 0
            head_sb = consts.tile([LIFT_P, head_cols], f16, name="head")
            nc.sync.dma_start(out=head_sb, in_=head_d[:, :])
            wp_sb = head_sb[:, 0:WPACK_COLS]
            ft_tiles = [head_sb[0:FT_P, WPACK_COLS:head_cols]]
            for d in range(1, len(DMASEG)):
                t = ftp.tile([FT_P, DMASEG[d]], f16, tag=f"ft{d}",
                             name=f"ft_sb{d}")
                nc.sync.dma_start(
                    out=t, in_=ft_d[:, DSTART[d] - DMASEG[0]:
                                    DSTART[d + 1] - DMASEG[0]])
                ft_tiles.append(t)

            def ft_slice(lo, hi):
                """(tile, tlo, thi) pieces covering ft cols [lo, hi)."""
                out = []
                for d in range(len(DMASEG)):
                    s, e = DSTART[d], DSTART[d + 1]
                    a, b = max(lo, s), min(hi, e)
                    if a < b:
                        out.append((ft_tiles[d], a - s, b - s))
                return out


            wlift_sb = wp_sb[0:FT_P, 0:LIFT_P]
            w1_sb = wp_sb[:, 120:200]          # [120, 80]
            w2_sb = wp_sb[0:10 * NBLK, 200:240]
            wro_sb = wp_sb[0:5 * NBLK, 240:248]
            bias_f32 = wp_sb[:, 248:256].bitcast(f32)   # [120, 4]
            b1 = bias_f32[0:10 * NBLK, 1:2]
            b2 = bias_f32[0:5 * NBLK, 2:3]
            bro = bias_f32[0:NBLK, 3:4]

            # absorb the wpack-DMA wait once per consumer engine
            dummy = consts.tile([LIFT_P, 4], f32)
            nc.scalar.copy(out=dummy, in_=bias_f32[:, 0:4])
            dummy2 = consts.tile([LIFT_P, 1], f32)
            nc.vector.tensor_copy(out=dummy2, in_=bias_f32[:, 0:1])

            t11 = treep.tile([LIFT_P, TOTG, SPLIT], f16, name="t11")
            xh12 = treep.tile([LIFT_P, TOTG, XHW], f16, name="xh12")
            wf = (treep.tile([LIFT_P, TOTG, NFOLD], f16, name="wfold")
                  if NFOLD else None)

            def lift_chunk(c):
                ps = psp.tile([LIFT_P, 1024], f32, tag="ps", name=f"ps_{c}")
                off = 0
                for (t, tlo, thi) in ft_slice(CSTART[c], CSTART[c + 1]):
                    nc.tensor.ldweights(weights=t[:, tlo:min(thi, tlo + 128)])
                    col = tlo
                    while col < thi:
                        n = min(512, thi - col)
                        nc.tensor.matmul(
                            ps[:, off:off + n],
                            lhsT=wlift_sb, rhs=t[:, col:col + n],
                            start=True, stop=True)
                        col += n
                        off += n
                return ps

            def reduce_chunk(c, ps):
                w = CHUNKS[c]
                ng = w // G
                g0 = CSTART[c] // G
                ps3 = ps[:, 0:w].rearrange("p (g i) -> p g i", i=G)
                nc.scalar.activation(
                    out=xh12[:, g0:g0 + ng, :], in_=ps3[:, :, SPLIT:G],
                    func=Relu, bias=0.0, scale=1.0)
                with nc.allow_low_precision(reason="fp16 partial sums"):
                    nc.vector.scalar_tensor_tensor(
                        out=t11[:, g0:g0 + ng, :], in0=ps3[:, :, 0:SPLIT],
                        scalar=0.0, in1=xh12[:, g0:g0 + ng, 0:SPLIT],
                        op0=Max, op1=Add)
                    if NFOLD:
                        nc.gpsimd.tensor_tensor(
                            out=wf[:, g0:g0 + ng, :],
                            in0=t11[:, g0:g0 + ng, 0:NFOLD],
                            in1=t11[:, g0:g0 + ng, NFOLD:2 * NFOLD],
                            op=Add)


            # ---- stage-2 pieces, interleaved into the chunk loop
            o_sb = s2p.tile([NBLK, TOTG], f32, name="osb")
            if OKV:
                kv_idx = consts.tile([128, 1], mybir.dt.int32, name="kvidx")
                nc.gpsimd.memset(kv_idx, 0)
                kv_sem = nc.alloc_semaphore(name="okv_sem")
                kv_prep_sem = nc.alloc_semaphore(name="okv_prep_sem")
            s2_state = {}

            def s2_p1(k):
                lo, hi = SEGS[k], SEGS[k + 1]
                w = hi - lo
                eng = S2ENG[min(k, len(S2ENG) - 1)]
                h1_ps = ps2p.tile([10 * NBLK, 512], f32, tag="s2",
                                  name=f"h1ps_{k}")
                if NFOLD and k < FOLD_SEGS:
                    rhss = ([wf[:, lo:hi, j] for j in range(NFOLD)] +
                            [t11[:, lo:hi, j]
                             for j in range(2 * NFOLD, SPLIT)] +
                            [xh12[:, lo:hi, j] for j in range(SPLIT, NPART)])
                else:
                    rhss = [(t11[:, lo:hi, j] if j < SPLIT
                             else xh12[:, lo:hi, j]) for j in range(NPART)]
                for j, rhs in enumerate(rhss):
                    nc.tensor.matmul(h1_ps[:, 0:w], lhsT=w1_sb, rhs=rhs,
                                     start=(j == 0), stop=(j == len(rhss) - 1))
                h1_sb = s2p.tile([10 * NBLK, 512], f16, tag=f"h1_{k}",
                                 name=f"h1sb_{k}")
                if eng == "A":
                    nc.scalar.activation(out=h1_sb[:, 0:w], in_=h1_ps[:, 0:w],
                                         func=Relu, bias=b1, scale=1.0)
                else:
                    nc.vector.tensor_scalar(
                        out=h1_sb[:, 0:w], in0=h1_ps[:, 0:w],
                        scalar1=b1, scalar2=0.0, op0=Add, op1=Max)
                s2_state[k] = h1_sb

            def s2_p2(k):
                lo, hi = SEGS[k], SEGS[k + 1]
                w = hi - lo
                eng = S2ENG[min(k, len(S2ENG) - 1)]
                h1_sb = s2_state[k]
                h2_ps = ps2p.tile([5 * NBLK, 512], f32, tag="s2",
                                  name=f"h2ps_{k}")
                nc.tensor.matmul(h2_ps[:, 0:w], lhsT=w2_sb, rhs=h1_sb[:, 0:w],
                                 start=True, stop=True)
                h2_sb = s2p.tile([5 * NBLK, 512], f16, tag=f"h2_{k}",
                                 name=f"h2sb_{k}")
                if eng == "A":
                    nc.scalar.activation(out=h2_sb[:, 0:w], in_=h2_ps[:, 0:w],
                                         func=Relu, bias=b2, scale=1.0)
                else:
                    nc.vector.tensor_scalar(
                        out=h2_sb[:, 0:w], in0=h2_ps[:, 0:w],
                        scalar1=b2, scalar2=0.0, op0=Add, op1=Max)
                s2_state[k] = h2_sb

            def s2_p3(k):
                lo, hi = SEGS[k], SEGS[k + 1]
                w = hi - lo
                h2_sb = s2_state[k]
                o_ps = ps2p.tile([NBLK, 512], f32, tag="s2", name=f"ops_{k}")
                nc.tensor.matmul(o_ps[:, 0:w], lhsT=wro_sb, rhs=h2_sb[:, 0:w],
                                 start=True, stop=True)
                nc.vector.tensor_scalar(out=o_sb[:, lo:hi], in0=o_ps[:, 0:w],
                                        scalar1=bro, scalar2=None, op0=Add)

            # chunk index after which each piece may be emitted
            gcum = 0
            lastch = []   # last chunk index for each segment
            cg = [0]
            for c in CHUNKS:
                gcum += c // G
                cg.append(gcum)
            for k in range(len(SEGS) - 1):
                need = SEGS[k + 1]
                lc = next(i for i in range(NCHUNK) if cg[i + 1] >= need)
                lastch.append(lc)
            pieces = []
            for k in range(len(SEGS) - 1):
                sk = -k if TAILREV else k
                pieces.append((lastch[k] + S2LAG, 1, sk, k))
                pieces.append((lastch[k] + S2LAG + 1, 2, sk, k))
                pieces.append((lastch[k] + S2LAG + 2, 3, sk, k))
            pieces.sort()
            pieces = [(a, b, d) for (a, b, _sk, d) in pieces]
            s2fn = {1: s2_p1, 2: s2_p2, 3: s2_p3}

            pi = 0
            for c in range(NCHUNK):
                ps = lift_chunk(c)
                reduce_chunk(c, ps)
                if OKV and c == NCHUNK - 2:
                    # descriptor prep runs on the idle Pool engine; the
                    # o_sb read dep transfers to the trigger below
                    nc.gpsimd.kv_writeback(
                        out_ap=out_d[:, :].rearrange(
                            "p (d n) -> () p d n", d=16),
                        in_ap=o_sb[:, :].rearrange(
                            "p (d n) -> p d () n", d=16),
                        ctx_idxs_ap=kv_idx, queue_num=1,
                        prepare_only=True, sem=kv_sem)
                while pi < len(pieces) and pieces[pi][0] <= c:
                    s2fn[pieces[pi][1]](pieces[pi][2])
                    pi += 1
            while pi < len(pieces):
                s2fn[pieces[pi][1]](pieces[pi][2])
                pi += 1
            if OKV:
                nc.gpsimd.trigger_dma(count=None, queue_num=1,
                                      signals_writable=[o_sb[:, :]])
            else:
                nc.sync.dma_start(out=out_d[:, :], in_=o_sb[:, :])

    if not nc.is_finalized():
        nc.finalize()
    return nc


def kernel(features, src, dst, W_lift, b_lift, W1, b1, W2, b2, W_ro, b_ro):
    global LAST_RESULT
    features = np.asarray(features, np.float32)
    src = np.asarray(src, np.int32)
    dst = np.asarray(dst, np.int32)
    W_lift = np.asarray(W_lift, np.float32)
    b_lift = np.asarray(b_lift, np.float32)
    W1 = np.asarray(W1, np.float32)
    b1 = np.asarray(b1, np.float32)
    W2 = np.asarray(W2, np.float32)
    b2 = np.asarray(b2, np.float32)
    W_ro = np.asarray(W_ro, np.float32)
    b_ro = np.asarray(b_ro, np.float32)

    if not _structured(src, dst):
        return _fallback_numpy(features, src, dst, W_lift, b_lift,
                               W1, b1, W2, b2, W_ro, b_ro)

    # feature-major per-slab layout with a ones row per slab: [80, 11264]
    ftn = (features.reshape(NCORES, NBLK, CHUNK_NODES, 9)
           .transpose(0, 1, 3, 2))           # [NCORES, NBLK, 9, CHUNK_NODES]
    ft = np.ones((NCORES, NBLK, FT_R, CHUNK_NODES), np.float16)
    ft[:, :, 0:9, :] = ftn.astype(np.float16)
    ft = ft.reshape(NCORES, FT_P, CHUNK_NODES)

    wpack = np.zeros((LIFT_P, WPACK_COLS), np.float16)
    wla = np.concatenate([W_lift, b_lift[None, :]], axis=0)   # [10, 15]
    wpack[0:FT_P, 0:LIFT_P] = _block_diag(wla, NBLK, np.float16)
    wpack[0:LIFT_P, 120:200] = _block_diag(W1, NBLK, np.float16)
    wpack[0:10 * NBLK, 200:240] = _block_diag((G * W2).astype(np.float32),
                                              NBLK, np.float16)
    wro_eff = W_ro.reshape(G, 5).sum(axis=0)
    for a in range(NBLK):
        wpack[5 * a:5 * (a + 1), 240 + a] = wro_eff

    bpack = np.zeros((LIFT_P, 4), np.float32)
    bpack[0:10 * NBLK, 1] = np.tile(b1, NBLK)
    bpack[0:5 * NBLK, 2] = np.tile(b2, NBLK)
    bpack[0:NBLK, 3] = float(b_ro[0])
    wpack[:, 248:256] = bpack.view(np.float16)

    if "nc" not in _cached:
        _cached["nc"] = _build_kernel()
    nc = _cached["nc"]

    from concourse import bass_utils

    hn = DMASEG[0]
    in_maps = []
    for c in range(NCORES):
        head = np.zeros((LIFT_P, WPACK_COLS + hn), np.float16)
        head[:, 0:WPACK_COLS] = wpack
        head[0:FT_P, WPACK_COLS:] = ft[c, :, 0:hn]
        in_maps.append({
            "head": head,
            "ft": np.ascontiguousarray(ft[c, :, hn:]),
        })

    trace = os.environ.get("KERNEL_TRACE", "0") == "1"
    res = None
    for attempt in range(4):
        try:
            res = bass_utils.run_bass_kernel_spmd(
                nc, in_maps, core_ids=list(range(NCORES)), trace=trace,
            )
            break
        except ModuleNotFoundError:
            trace = False
        except Exception as e:  # noqa: BLE001
            if attempt == 3 or "UNRECOVERABLE" not in str(e).upper():
                raise
            import time
            time.sleep(15)
    LAST_RESULT = res

    out = np.concatenate([r["out"].reshape(-1) for r in res.results])
    return np.ascontiguousarray(out.reshape(B, 1).astype(np.float32))

